# revision 14
# baseline (speedup 1.0000x reference)
"""Trainium2 Bass kernel for nn_RadialModel (forward NUFFT, radial MRI).

Per-core (1 frame, all 8 coils):
  1. coil multiply (DVE, bf16 out)
  2. DFT via bf16 PE matmuls (two stages), stage-1 for ALL coils first,
     then stage-2 vt-major (order 3,0,1,2) so table regions finish
     progressively and gathers start during the DFT.
  3. grid rows staged in SBUF as 526-cell extended rows (bf16), stored
     to DRAM table T4 with FOUR 2-cell-shifted copies row-interleaved:
     unit(r, c, u) = (r*4 + c)*65 + u; one 256B unit = 8 cells x 16 cri.
  4. interpolation via batched dma_gather: points HOST-SORTED by grid
     row; per point-tile (1024 sorted points) 6 calls x 1024 idxs fetch
     one 256B unit per (point, row-tap); weighted reduce on DVE with an
     8-tap x-kernel (poly forced to 0 outside support); tree-add
     reductions (contiguous-run reads). Gather indices and fractional
     metadata precomputed on the host from k.
  5. sqrt(w) scale + store

Sharding: one frame (nt) per NeuronCore, 8 cores.
"""
import numpy as np

import concourse.bass as bass
import concourse.bacc as bacc
import concourse.mybir as mybir
import concourse.tile as tile
from concourse.bass_utils import run_bass_kernel_spmd

F32 = mybir.dt.float32
I16 = mybir.dt.int16
BF16 = mybir.dt.bfloat16
AX = mybir.AxisListType
OP = mybir.AluOpType

IM = 256
G = 512
J = 6
ALPHA = 2.34 * J
TWO_PI = 2.0 * np.pi
NT, NC, K = 8, 8, 16384
CELL = NC * 2            # cri values per grid cell = 16
ROWC = 526               # extended row cells: [2 wrap][512][12 wrap]
UPR = 65                 # 256B units per stored row copy (520 cells)
NCOPY = 4                # 2-cell-shifted row copies
UNITS = 517 * NCOPY * UPR
WIN = 32768              # int16-addressable units per gather window
NTILE = 16
NB = 6                   # y taps
NS = 8                   # x taps (8 fetched cells)
DEG = 8                  # KB poly degree (w = t * p(t))

# table-store groups each point tile's gathers must wait for
_NEEDS = {m: (["vt0", "h3"] if m <= 1 else
              ["vt0", "vt1"] if m <= 5 else
              ["vt1", "vt2"] if m <= 9 else
              ["vt2", "vt3"] if m <= 13 else
              ["vt3", "h0"]) for m in range(NTILE)}
# emission order: tiles whose stores complete earliest first
_MORDER = [0, 1, 14, 15, 2, 3, 4, 5, 6, 7, 8, 9, 10, 11, 12, 13]


def _base_unit(m):
    return int(np.clip(260 * (32 * m - 40), 0, UNITS - WIN))


# ---------------------------------------------------------------- host consts
def _host_consts():
    f = (np.arange(IM) - IM // 2) / G
    z = (np.pi * J * f) ** 2 - ALPHA ** 2
    s = np.sqrt(np.abs(z))
    val = np.where(z < 0, np.sinh(s) / np.maximum(s, 1e-12), np.sinc(s / np.pi))
    ftkb = (J / np.i0(ALPHA)) * val
    scal = 1.0 / ftkb
    u = np.arange(G)[:, None].astype(np.float64)
    xp = np.arange(IM)[None, :].astype(np.float64)
    A = np.exp(1j * np.pi * u / 2 - 2j * np.pi * u * xp / G) * scal[None, :] / np.sqrt(G)
    art = np.ascontiguousarray(A.T.real, dtype=np.float32)
    ait = np.ascontiguousarray(A.T.imag, dtype=np.float32)
    aitn = np.ascontiguousarray(-A.T.imag, dtype=np.float32)
    n = 512
    x = (1 - np.cos(np.pi * (np.arange(n) + 0.5) / n)) / 2
    w = np.i0(ALPHA * np.sqrt(x)) / np.i0(ALPHA)
    V = np.stack([x ** d for d in range(1, DEG + 1)], axis=1)
    c, *_ = np.linalg.lstsq(V, w, rcond=None)
    err = np.abs(V @ c - w).max()
    assert err < 5e-5, err
    return art, ait, aitn, c.astype(np.float64)


_ART, _AIT, _AITN, _PC = _host_consts()
import ml_dtypes as _mld
_ARTB = np.ascontiguousarray(_ART.astype(_mld.bfloat16))
_AITB = np.ascontiguousarray(_AIT.astype(_mld.bfloat16))
_AITNB = np.ascontiguousarray(_AITN.astype(_mld.bfloat16))


# ---------------------------------------------------------------- bass build
def build_bass(debug=False):
    nc = bacc.Bacc(num_swdge_queues=4)

    x_in = nc.declare_dram_parameter("x", [2, IM, IM], F32, isOutput=False)
    c_in = nc.declare_dram_parameter("coil", [NC, 2, IM, IM], F32, isOutput=False)
    w_in = nc.declare_dram_parameter("wr", [128, NTILE * 128], F32, isOutput=False)
    i_in = nc.declare_dram_parameter("idxr", [128, NTILE * NB * 64], I16,
                                     isOutput=False)
    m_in = nc.declare_dram_parameter("meta", [128, NTILE * 8 * 2], F32,
                                     isOutput=False)
    art_in = nc.declare_dram_parameter("art", [IM, G], BF16, isOutput=False)
    ait_in = nc.declare_dram_parameter("ait", [IM, G], BF16, isOutput=False)
    aitn_in = nc.declare_dram_parameter("aitn", [IM, G], BF16, isOutput=False)
    y_out = nc.declare_dram_parameter("yr", [128, NTILE * 128], F32, isOutput=True)

    T4 = nc.dram_tensor("T4", [UNITS, 128], BF16)

    PC = _PC

    def horner_w(pool, t, shape, tag):
        acc = pool.tile(shape, F32, tag=tag)
        nc.vector.tensor_scalar(
            out=acc[:], in0=t, scalar1=float(PC[DEG - 1]),
            scalar2=float(PC[DEG - 2]), op0=OP.mult, op1=OP.add,
        )
        for d_ in range(DEG - 3, -1, -1):
            nc.vector.tensor_tensor(out=acc[:], in0=acc[:], in1=t, op=OP.mult)
            nc.vector.tensor_scalar_add(acc[:], acc[:], float(PC[d_]))
        nc.vector.tensor_tensor(out=acc[:], in0=acc[:], in1=t, op=OP.mult)
        return acc

    with tile.TileContext(nc) as tc:
        with (
            tc.tile_pool(name="const", bufs=1) as constp,
            tc.tile_pool(name="work", bufs=1) as workp,
            tc.tile_pool(name="ctile", bufs=2) as coilp,
            tc.tile_pool(name="mtile", bufs=3) as mp,
            tc.tile_pool(name="bt", bufs=1) as btp,
            tc.tile_pool(name="stg", bufs=1) as stgp,
            tc.tile_pool(name="patch", bufs=2) as patchp,
            tc.tile_pool(name="cmb", bufs=1) as cmbp,
            tc.tile_pool(name="ps1", bufs=4, space="PSUM") as ps1,
            tc.tile_pool(name="ps2", bufs=4, space="PSUM") as ps2,
        ):
            # ---------------- constants (bf16 DFT matrices) ----------------
            art = []
            for name, asrc in (("art", art_in), ("ait", ait_in), ("aitn", aitn_in)):
                ts_ = []
                for xt in range(2):
                    tb = constp.tile([128, G], BF16, tag=f"{name}{xt}")
                    nc.sync.dma_start(out=tb[:], in_=asrc[xt * 128:(xt + 1) * 128, :])
                    ts_.append(tb)
                art.append(ts_)
            artT, aitT, aitnT = art

            # ---------------- res buffer ----------------
            res = workp.tile([128, NTILE * 128], F32, tag="res")

            # x image tiles (bf16, persist across coils)
            xts = []
            for xt in range(2):
                xf = workp.tile([128, 2 * IM], F32, tag=f"xt{xt}f")
                nc.sync.dma_start(
                    out=xf[:],
                    in_=x_in[:, xt * 128:(xt + 1) * 128, :]
                    .rearrange("ri x y -> x ri y"),
                )
                xts.append(xf)

            stgs = []
            for vt in range(4):
                stg = stgp.tile([128, ROWC * CELL], BF16, tag=f"stg{vt}")
                stgs.append(stg)

            # ======== stage 1 for ALL coils (bf16 bt) ========
            bts = {}
            for c in range(NC):
                mt = []
                for xt in range(2):
                    ct = coilp.tile([128, 2 * IM], F32, tag="ct")
                    nc.sync.dma_start(
                        out=ct[:],
                        in_=c_in[c, :, xt * 128:(xt + 1) * 128, :]
                        .rearrange("ri x y -> x ri y"),
                    )
                    xt_t = xts[xt]
                    m_ = mp.tile([128, 2 * IM], BF16, tag="m")
                    xr, xi = xt_t[:, 0:IM], xt_t[:, IM:2 * IM]
                    cr, ci = ct[:, 0:IM], ct[:, IM:2 * IM]
                    mr, mi = m_[:, 0:IM], m_[:, IM:2 * IM]
                    t1 = mp.tile([128, IM], F32, tag="cm1")
                    t2 = mp.tile([128, IM], F32, tag="cm2")
                    nc.gpsimd.tensor_mul(t1[:], xr, cr)
                    nc.gpsimd.tensor_mul(t2[:], xi, ci)
                    nc.gpsimd.tensor_sub(mr, t1[:], t2[:])
                    nc.gpsimd.tensor_mul(t1[:], xr, ci)
                    nc.gpsimd.tensor_mul(t2[:], xi, cr)
                    nc.gpsimd.tensor_add(mi, t1[:], t2[:])
                    mt.append(m_)
                for yt in range(2):
                    pr = ps1.tile([128, G], F32, tag="psa")
                    pi = ps1.tile([128, G], F32, tag="psa")
                    for xt in range(2):
                        mrb = mt[xt][:, yt * 128:yt * 128 + 128]
                        mib = mt[xt][:, IM + yt * 128:IM + yt * 128 + 128]
                        st = xt == 0
                        sp = xt == 1
                        nc.tensor.matmul(pr[:], mrb, artT[xt][:], start=st, stop=False)
                        nc.tensor.matmul(pi[:], mrb, aitT[xt][:], start=st, stop=False)
                        nc.tensor.matmul(pr[:], mib, aitnT[xt][:], start=False, stop=sp)
                        nc.tensor.matmul(pi[:], mib, artT[xt][:], start=False, stop=sp)
                    btr = btp.tile([128, G], BF16, tag=f"bt{c}r{yt}")
                    bti = btp.tile([128, G], BF16, tag=f"bt{c}i{yt}")
                    nc.scalar.copy(out=btr[:], in_=pr[:])
                    nc.scalar.copy(out=bti[:], in_=pi[:])
                    bts[(0, yt, c)] = btr
                    bts[(1, yt, c)] = bti

            # ======== stage 2 vt-major; stores released per vt ========
            store_groups = {"vt0": [], "vt1": [], "vt2": [], "vt3": [],
                            "h0": [], "h3": []}
            T4v = T4[:].rearrange("(r c u) e -> r c (u e)", c=NCOPY, u=UPR)
            for vt in (3, 0, 1, 2):
                stg = stgs[vt]
                stg3 = stg[:].rearrange("p (u e) -> p u e", e=CELL)
                for c in range(NC):
                    gr = ps2.tile([128, G], F32, tag="psb")
                    gi = ps2.tile([128, G], F32, tag="psb")
                    for yt in range(2):
                        av = artT[yt][:, vt * 128:(vt + 1) * 128]
                        aiv = aitT[yt][:, vt * 128:(vt + 1) * 128]
                        ainv = aitnT[yt][:, vt * 128:(vt + 1) * 128]
                        btr = bts[(0, yt, c)]
                        bti = bts[(1, yt, c)]
                        st = yt == 0
                        sp = yt == 1
                        nc.tensor.matmul(gr[:], av, btr[:], start=st, stop=False)
                        nc.tensor.matmul(gi[:], aiv, btr[:], start=st, stop=False)
                        nc.tensor.matmul(gr[:], ainv, bti[:], start=False, stop=sp)
                        nc.tensor.matmul(gi[:], av, bti[:], start=False, stop=sp)
                    nc.scalar.copy(
                        out=stg3[:, 2:2 + G, 2 * c:2 * c + 1], in_=gr[:].unsqueeze(2)
                    )
                    deng = nc.scalar.copy if c < 2 else (
                        lambda out, in_: nc.vector.tensor_copy(out=out, in_=in_))
                    deng(
                        out=stg3[:, 2:2 + G, 2 * c + 1:2 * c + 2],
                        in_=gi[:].unsqueeze(2),
                    )
                # wrap halo cells, then store 4 shifted copies
                nc.vector.tensor_copy(
                    out=stg[:, 0:2 * CELL], in_=stg[:, 512 * CELL:514 * CELL]
                )
                nc.vector.tensor_copy(
                    out=stg[:, 514 * CELL:526 * CELL], in_=stg[:, 2 * CELL:14 * CELL]
                )
                r0 = vt * 128 + 2
                for cc in range(NCOPY):
                    store_groups[f"vt{vt}"].append(nc.sync.dma_start(
                        out=T4v[r0:r0 + 128, cc, :],
                        in_=stg[:, 2 * cc * CELL:(2 * cc + 520) * CELL],
                    ))
                if vt == 0:
                    for cc in range(NCOPY):
                        store_groups["h0"].append(nc.sync.dma_start(
                            out=T4v[514:517, cc, :],
                            in_=stg[0:3, 2 * cc * CELL:(2 * cc + 520) * CELL],
                        ))
                if vt == 3:
                    for cc in range(NCOPY):
                        store_groups["h3"].append(nc.sync.dma_start(
                            out=T4v[0:2, cc, :],
                            in_=stg[126:128, 2 * cc * CELL:(2 * cc + 520) * CELL],
                        ))

            # ---------------- w load + sqrt ----------------
            wsq = workp.tile([128, NTILE * 128], F32, tag="wsq")
            nc.sync.dma_start(out=wsq[:], in_=w_in[:])
            nc.scalar.activation(
                out=wsq[:], in_=wsq[:],
                func=mybir.ActivationFunctionType.Sqrt,
            )

            # ---------------- idx / meta loads ----------------
            idx_rep = workp.tile([128, NTILE * NB * 64], I16, tag="idxrep")
            nc.sync.dma_start(out=idx_rep[:], in_=i_in[:])
            meta_all = workp.tile([128, NTILE * 8 * 2], F32, tag="meta")
            nc.sync.dma_start(out=meta_all[:], in_=m_in[:])

            # ======== weights (all tiles) ========
            exv = bass.AP(
                meta_all[:].tensor, meta_all[:].offset,
                [meta_all[:].ap[0], [2, NTILE * 8], [0, NS]],
            )
            sconst = constp.tile([128, NS], F32, tag="sconst")
            for s_ in range(NS):
                nc.vector.memset(sconst[:, s_:s_ + 1], float(2 - s_))
            ux = workp.tile([128, NTILE * 8 * NS], F32, tag="ux")
            nc.vector.tensor_tensor(
                out=ux[:].rearrange("p (mc s) -> p mc s", s=NS),
                in0=exv, in1=bass.AP(
                    sconst[:].tensor, sconst[:].offset,
                    [sconst[:].ap[0], [0, NTILE * 8], [1, NS]],
                ),
                op=OP.add,
            )
            nc.vector.tensor_mul(ux[:], ux[:], ux[:])
            nc.vector.tensor_scalar(
                out=ux[:], in0=ux[:], scalar1=float(-1.0 / 9.0), scalar2=1.0,
                op0=OP.mult, op1=OP.add,
            )
            nc.vector.tensor_scalar_max(ux[:], ux[:], 0.0)
            wxall = horner_w(workp, ux[:], [128, NTILE * 8 * NS], "wxall")

            eyv = bass.AP(
                meta_all[:].tensor, meta_all[:].offset + 1,
                [meta_all[:].ap[0], [2, NTILE * 8], [0, NB]],
            )
            bconst = constp.tile([128, NB], F32, tag="bconst")
            for b in range(NB):
                nc.vector.memset(bconst[:, b:b + 1], float(2 - b))
            uy = workp.tile([128, NTILE * 8 * NB], F32, tag="uy")
            nc.vector.tensor_tensor(
                out=uy[:].rearrange("p (mc b) -> p mc b", b=NB),
                in0=eyv, in1=bass.AP(
                    bconst[:].tensor, bconst[:].offset,
                    [bconst[:].ap[0], [0, NTILE * 8], [1, NB]],
                ),
                op=OP.add,
            )
            nc.vector.tensor_mul(uy[:], uy[:], uy[:])
            nc.vector.tensor_scalar(
                out=uy[:], in0=uy[:], scalar1=float(-1.0 / 9.0), scalar2=1.0,
                op0=OP.mult, op1=OP.add,
            )
            nc.vector.tensor_scalar_max(uy[:], uy[:], 0.0)
            wyall = horner_w(workp, uy[:], [128, NTILE * 8 * NB], "wyall")


            # ======== gather + combine (release order _MORDER) ========
            for m in _MORDER:
                patch = patchp.tile([128, NB * 8 * 128], BF16, tag="patch")
                gathers = []
                for b in range(NB):
                    gi_ = nc.gpsimd.dma_gather(
                        out_ap=patch[:, b * 1024:(b + 1) * 1024].rearrange(
                            "p (ch e) -> p ch e", e=128),
                        in_ap=T4[_base_unit(m):_base_unit(m) + WIN, :],
                        idxs_ap=idx_rep[:, (m * NB + b) * 64:(m * NB + b + 1) * 64],
                        num_idxs=1024,
                        num_idxs_reg=1024,
                        elem_size=128,
                        queue_num=(m * NB + b) % 4,
                    )
                    gathers.append(gi_)
                for gi_ in gathers:
                    for grp in _NEEDS[m]:
                        for si in store_groups[grp]:
                            tile.add_dep_helper(gi_.ins, si.ins, reason="T RAW")
                # x-weights read directly (C, s, cr-broadcast) like the ymult
                wxs = bass.AP(
                    wxall[:].tensor, wxall[:].offset + m * 8 * NS,
                    [wxall[:].ap[0], [NS, 8], [1, NS], [0, CELL]],
                )
                rb = cmbp.tile([128, NB * 128], F32, tag="rb")
                for b in range(NB):
                    wp_ = cmbp.tile([128, 1024], BF16, tag="wp")
                    nc.vector.tensor_tensor(
                        out=wp_[:].rearrange("p (c s e) -> p c s e", s=NS, e=CELL),
                        in0=patch[:, b * 1024:(b + 1) * 1024].rearrange(
                            "p (c s e) -> p c s e", s=NS, e=CELL),
                        in1=wxs, op=OP.mult,
                    )
                    # tree-add s-reduce (contiguous 64/32-elem runs)
                    h1 = cmbp.tile([128, 512], BF16, tag="h1")
                    v0 = bass.AP(wp_[:].tensor, wp_[:].offset,
                                 [wp_[:].ap[0], [128, 8], [1, 64]])
                    v1 = bass.AP(wp_[:].tensor, wp_[:].offset + 64,
                                 [wp_[:].ap[0], [128, 8], [1, 64]])
                    h1v = h1[:].rearrange("p (ch e) -> p ch e", e=64)
                    nc.vector.tensor_tensor(out=h1v, in0=v0, in1=v1, op=OP.add)
                    h2 = cmbp.tile([128, 256], BF16, tag="h2")
                    w0 = bass.AP(h1[:].tensor, h1[:].offset,
                                 [h1[:].ap[0], [64, 8], [1, 32]])
                    w1 = bass.AP(h1[:].tensor, h1[:].offset + 32,
                                 [h1[:].ap[0], [64, 8], [1, 32]])
                    h2v = h2[:].rearrange("p (ch e) -> p ch e", e=32)
                    nc.vector.tensor_tensor(out=h2v, in0=w0, in1=w1, op=OP.add)
                    rb3 = rb[:, b * 128:(b + 1) * 128].rearrange(
                        "p (ch e) -> p ch e", e=16)
                    z0 = bass.AP(h2[:].tensor, h2[:].offset,
                                 [h2[:].ap[0], [32, 8], [1, 16]])
                    z1 = bass.AP(h2[:].tensor, h2[:].offset + 16,
                                 [h2[:].ap[0], [32, 8], [1, 16]])
                    nc.vector.tensor_tensor(out=rb3, in0=z0, in1=z1, op=OP.add)
                # vb[P, (b, C, cr)] = rb * wy ; tree-add over b
                vb = cmbp.tile([128, NB * 128], F32, tag="vb")
                rbv = bass.AP(
                    rb[:].tensor, rb[:].offset,
                    [rb[:].ap[0], [128, NB], [CELL, 8], [1, CELL]],
                )
                wys = bass.AP(
                    wyall[:].tensor, wyall[:].offset + m * 8 * NB,
                    [wyall[:].ap[0], [1, NB], [NB, 8], [0, CELL]],
                )
                nc.vector.tensor_tensor(
                    out=vb[:].rearrange("p (b ch e) -> p b ch e", ch=8, e=CELL),
                    in0=rbv, in1=wys, op=OP.mult,
                )
                h3_ = cmbp.tile([128, 384], F32, tag="h3t")
                nc.vector.tensor_tensor(
                    out=h3_[:], in0=vb[:, 0:384], in1=vb[:, 384:768], op=OP.add
                )
                h4 = cmbp.tile([128, 128], F32, tag="h4t")
                nc.vector.tensor_tensor(
                    out=h4[:], in0=h3_[:, 0:128], in1=h3_[:, 128:256], op=OP.add
                )
                nc.vector.tensor_tensor(
                    out=res[:, m * 128:(m + 1) * 128], in0=h4[:],
                    in1=h3_[:, 256:384], op=OP.add,
                )

            # ======== sqrt(w) scale + store ========
            nc.vector.tensor_mul(res[:], res[:], wsq[:])
            nc.sync.dma_start(out=y_out[:], in_=res[:])

    nc.compile()
    return nc


_NC_CACHE = None


def _get_nc():
    global _NC_CACHE
    if _NC_CACHE is None:
        _NC_CACHE = build_bass()
    return _NC_CACHE


# ---------------------------------------------------------------- host glue
def _point_map():
    P = np.arange(128)
    m = np.arange(NTILE)
    C = np.arange(8)
    return (m[None, :, None] * 1024 + (P % 16)[:, None, None] * 64
            + C[None, None, :] * 8 + (P // 16)[:, None, None])


_PMAP = _point_map()
_BASES = np.array([_base_unit(m) for m in range(NTILE)], dtype=np.int64)


def _host_idx_meta(kt):
    kv = np.asarray(kt, dtype=np.float32)
    gx0 = kv * np.float32(G / TWO_PI)
    gxy = np.where(gx0 < 0, gx0 + np.float32(G), gx0).astype(np.float32)
    gm3 = (gxy - np.float32(3.0)).astype(np.float32)
    fl = np.round((gm3 - np.float32(0.498046875)).astype(np.float32))
    rr = (gm3 - fl).astype(np.float32)
    fli = fl.astype(np.int64)
    perm = np.argsort(fli[1], kind="stable").astype(np.int64)

    q0 = 3 + fli[0]
    u8, m8 = q0 // 8, q0 % 8
    c4, d2 = m8 // 2, m8 % 2
    xunit = 65 * c4 + u8
    row0 = fli[1] + 3

    sp = perm
    m_of_s = np.arange(K) // 1024
    flat0 = 260 * row0[sp] + xunit[sp] - _BASES[m_of_s]
    s_grid = (np.arange(16)[:, None, None] * 64
              + np.arange(NTILE)[None, :, None] * 1024
              + np.arange(64)[None, None, :])
    f0 = flat0[s_grid]
    b_off = (260 * np.arange(NB))[None, :, None]
    idxw = f0[:, :, None, :] + b_off[None]
    idxw = np.clip(idxw, 0, WIN - 1).astype(np.int16)
    idxw = idxw.reshape(16, NTILE * NB * 64)
    idx_rep = np.ascontiguousarray(np.tile(idxw, (8, 1)))

    kmap = perm[_PMAP]
    ex = (rr[0] + d2.astype(np.float32))[kmap]
    ey = rr[1][kmap]
    meta = np.stack([ex, ey], axis=-1).astype(np.float32)
    return perm, idx_rep, np.ascontiguousarray(meta.reshape(128, NTILE * 8 * 2))


def make_in_maps(x, k, coil_sensitivities, w):
    in_maps = []
    coil0 = np.ascontiguousarray(coil_sensitivities[0], dtype=np.float32)
    perms = []
    for t in range(NT):
        perm, idx_rep, meta = _host_idx_meta(np.asarray(k[t], dtype=np.float32))
        perms.append(perm)
        kmap = perm[_PMAP]
        wt = np.asarray(w[t], dtype=np.float32)
        wr = wt[:, :, kmap]
        wr = np.ascontiguousarray(
            wr.transpose(2, 3, 4, 0, 1).reshape(128, NTILE * 128))
        in_maps.append({
            "x": np.ascontiguousarray(x[t], dtype=np.float32),
            "coil": coil0,
            "wr": wr,
            "idxr": idx_rep,
            "meta": meta,
            "art": _ARTB, "ait": _AITB, "aitn": _AITNB,
        })
    return in_maps, perms


def _unshuffle_y(yr, perm):
    v = yr.reshape(128, NTILE, 8, NC, 2)
    kmap = perm[_PMAP]
    out = np.empty((NC, 2, K), dtype=np.float32)
    out[:, :, kmap] = v.transpose(3, 4, 0, 1, 2)
    return out


def run(x, k, coil_sensitivities, w, trace=False, **spmd_kwargs):
    nc = _get_nc()
    in_maps, perms = make_in_maps(x, k, coil_sensitivities, w)
    r = run_bass_kernel_spmd(nc, in_maps, list(range(NT)), trace=trace, **spmd_kwargs)
    y = np.stack(
        [_unshuffle_y(r.results[t]["yr"], perms[t]) for t in range(NT)], axis=0
    )
    return y.astype(np.float32), r


def kernel(x, k, coil_sensitivities, w):
    y, _ = run(x, k, coil_sensitivities, w, trace=False)
    return y


# revision 16
# speedup vs baseline: 1.0642x; 1.0642x over previous
"""Trainium2 Bass kernel for nn_RadialModel (forward NUFFT, radial MRI).

Per-core (1 frame, all 8 coils):
  1. coil multiply (DVE, bf16 out)
  2. DFT via bf16 PE matmuls (two stages), stage-1 for ALL coils first,
     then stage-2 vt-major (order 3,0,1,2) so table regions finish
     progressively and gathers start during the DFT.
  3. grid rows staged in SBUF as 526-cell extended rows (bf16), stored
     to DRAM table T4 with FOUR 2-cell-shifted copies row-interleaved:
     unit(r, c, u) = (r*4 + c)*65 + u; one 256B unit = 8 cells x 16 cri.
  4. interpolation via batched dma_gather: points HOST-SORTED by grid
     row; per point-tile (1024 sorted points) 6 calls x 1024 idxs fetch
     one 256B unit per (point, row-tap); weighted reduce on DVE with an
     8-tap x-kernel (poly forced to 0 outside support); tree-add
     reductions (contiguous-run reads). Gather indices and fractional
     metadata precomputed on the host from k.
  5. sqrt(w) scale + store

Sharding: one frame (nt) per NeuronCore, 8 cores.
"""
import numpy as np

import concourse.bass as bass
import concourse.bacc as bacc
import concourse.mybir as mybir
import concourse.tile as tile
from concourse.bass_utils import run_bass_kernel_spmd

F32 = mybir.dt.float32
I16 = mybir.dt.int16
BF16 = mybir.dt.bfloat16
AX = mybir.AxisListType
OP = mybir.AluOpType

IM = 256
G = 512
J = 6
ALPHA = 2.34 * J
TWO_PI = 2.0 * np.pi
NT, NC, K = 8, 8, 16384
CELL = NC * 2            # cri values per grid cell = 16
ROWC = 526               # extended row cells: [2 wrap][512][12 wrap]
UPR = 65                 # 256B units per stored row copy (520 cells)
NCOPY = 4                # 2-cell-shifted row copies
UNITS = 517 * NCOPY * UPR
WIN = 32768              # int16-addressable units per gather window
NTILE = 16
NB = 6                   # y taps
NS = 8                   # x taps (8 fetched cells)
DEG = 8                  # KB poly degree (w = t * p(t))

# table-store groups each point tile's gathers must wait for
_NEEDS = {m: (["vt0", "h3"] if m <= 1 else
              ["vt0", "vt1"] if m <= 5 else
              ["vt1", "vt2"] if m <= 9 else
              ["vt2", "vt3"] if m <= 13 else
              ["vt3", "h0"]) for m in range(NTILE)}
# emission order: tiles whose stores complete earliest first
_MORDER = [0, 1, 14, 15, 2, 3, 4, 5, 6, 7, 8, 9, 10, 11, 12, 13]


def _base_unit(m):
    return int(np.clip(260 * (32 * m - 40), 0, UNITS - WIN))


# ---------------------------------------------------------------- host consts
def _host_consts():
    f = (np.arange(IM) - IM // 2) / G
    z = (np.pi * J * f) ** 2 - ALPHA ** 2
    s = np.sqrt(np.abs(z))
    val = np.where(z < 0, np.sinh(s) / np.maximum(s, 1e-12), np.sinc(s / np.pi))
    ftkb = (J / np.i0(ALPHA)) * val
    scal = 1.0 / ftkb
    u = np.arange(G)[:, None].astype(np.float64)
    xp = np.arange(IM)[None, :].astype(np.float64)
    A = np.exp(1j * np.pi * u / 2 - 2j * np.pi * u * xp / G) * scal[None, :] / np.sqrt(G)
    art = np.ascontiguousarray(A.T.real, dtype=np.float32)
    ait = np.ascontiguousarray(A.T.imag, dtype=np.float32)
    aitn = np.ascontiguousarray(-A.T.imag, dtype=np.float32)
    n = 512
    x = (1 - np.cos(np.pi * (np.arange(n) + 0.5) / n)) / 2
    w = np.i0(ALPHA * np.sqrt(x)) / np.i0(ALPHA)
    V = np.stack([x ** d for d in range(1, DEG + 1)], axis=1)
    c, *_ = np.linalg.lstsq(V, w, rcond=None)
    err = np.abs(V @ c - w).max()
    assert err < 5e-5, err
    return art, ait, aitn, c.astype(np.float64)


_ART, _AIT, _AITN, _PC = _host_consts()
import ml_dtypes as _mld
_ARTB = np.ascontiguousarray(_ART.astype(_mld.bfloat16))
_AITB = np.ascontiguousarray(_AIT.astype(_mld.bfloat16))
_AITNB = np.ascontiguousarray(_AITN.astype(_mld.bfloat16))


# ---------------------------------------------------------------- bass build
def build_bass(debug=False):
    nc = bacc.Bacc(num_swdge_queues=4)

    x_in = nc.declare_dram_parameter("x", [2, IM, IM], F32, isOutput=False)
    c_in = nc.declare_dram_parameter("coil", [NC, 2, IM, IM], F32, isOutput=False)
    w_in = nc.declare_dram_parameter("wr", [128, NTILE * 128], F32, isOutput=False)
    i_in = nc.declare_dram_parameter("idxr", [128, NTILE * NB * 64], I16,
                                     isOutput=False)
    m_in = nc.declare_dram_parameter("meta", [128, NTILE * 8 * 2], F32,
                                     isOutput=False)
    art_in = nc.declare_dram_parameter("art", [IM, G], BF16, isOutput=False)
    ait_in = nc.declare_dram_parameter("ait", [IM, G], BF16, isOutput=False)
    aitn_in = nc.declare_dram_parameter("aitn", [IM, G], BF16, isOutput=False)
    y_out = nc.declare_dram_parameter("yr", [128, NTILE * 128], F32, isOutput=True)

    T4 = nc.dram_tensor("T4", [UNITS, 128], BF16)

    PC = _PC

    def horner_w(pool, t, shape, tag):
        acc = pool.tile(shape, F32, tag=tag)
        nc.vector.tensor_scalar(
            out=acc[:], in0=t, scalar1=float(PC[DEG - 1]),
            scalar2=float(PC[DEG - 2]), op0=OP.mult, op1=OP.add,
        )
        for d_ in range(DEG - 3, -1, -1):
            nc.vector.tensor_tensor(out=acc[:], in0=acc[:], in1=t, op=OP.mult)
            nc.vector.tensor_scalar_add(acc[:], acc[:], float(PC[d_]))
        nc.vector.tensor_tensor(out=acc[:], in0=acc[:], in1=t, op=OP.mult)
        return acc

    with tile.TileContext(nc) as tc:
        with (
            tc.tile_pool(name="const", bufs=1) as constp,
            tc.tile_pool(name="work", bufs=1) as workp,
            tc.tile_pool(name="ctile", bufs=2) as coilp,
            tc.tile_pool(name="mtile", bufs=3) as mp,
            tc.tile_pool(name="bt", bufs=1) as btp,
            tc.tile_pool(name="stg", bufs=1) as stgp,
            tc.tile_pool(name="patch", bufs=2) as patchp,
            tc.tile_pool(name="cmb", bufs=1) as cmbp,
            tc.tile_pool(name="ps1", bufs=4, space="PSUM") as ps1,
            tc.tile_pool(name="ps2", bufs=4, space="PSUM") as ps2,
        ):
            # ---------------- constants (bf16 DFT matrices) ----------------
            art = []
            for name, asrc in (("art", art_in), ("ait", ait_in), ("aitn", aitn_in)):
                ts_ = []
                for xt in range(2):
                    tb = constp.tile([128, G], BF16, tag=f"{name}{xt}")
                    nc.sync.dma_start(out=tb[:], in_=asrc[xt * 128:(xt + 1) * 128, :])
                    ts_.append(tb)
                art.append(ts_)
            artT, aitT, aitnT = art

            # ---------------- res buffer ----------------
            res = workp.tile([128, NTILE * 128], F32, tag="res")

            # x image tiles (bf16, persist across coils)
            xts = []
            for xt in range(2):
                xf = workp.tile([128, 2 * IM], F32, tag=f"xt{xt}f")
                nc.sync.dma_start(
                    out=xf[:],
                    in_=x_in[:, xt * 128:(xt + 1) * 128, :]
                    .rearrange("ri x y -> x ri y"),
                )
                xts.append(xf)

            stgs = []
            for vt in range(4):
                stg = stgp.tile([128, ROWC * CELL], BF16, tag=f"stg{vt}")
                stgs.append(stg)

            # ======== stage 1 for ALL coils (bf16 bt) ========
            bts = {}
            for c in range(NC):
                mt = []
                for xt in range(2):
                    ct = coilp.tile([128, 2 * IM], F32, tag="ct")
                    nc.sync.dma_start(
                        out=ct[:],
                        in_=c_in[c, :, xt * 128:(xt + 1) * 128, :]
                        .rearrange("ri x y -> x ri y"),
                    )
                    xt_t = xts[xt]
                    m_ = mp.tile([128, 2 * IM], BF16, tag="m")
                    xr, xi = xt_t[:, 0:IM], xt_t[:, IM:2 * IM]
                    cr, ci = ct[:, 0:IM], ct[:, IM:2 * IM]
                    mr, mi = m_[:, 0:IM], m_[:, IM:2 * IM]
                    t1 = mp.tile([128, IM], F32, tag="cm1")
                    t2 = mp.tile([128, IM], F32, tag="cm2")
                    nc.vector.tensor_mul(t1[:], xr, cr)
                    nc.vector.tensor_mul(t2[:], xi, ci)
                    nc.vector.tensor_sub(mr, t1[:], t2[:])
                    nc.vector.tensor_mul(t1[:], xr, ci)
                    nc.vector.tensor_mul(t2[:], xi, cr)
                    nc.vector.tensor_add(mi, t1[:], t2[:])
                    mt.append(m_)
                for yt in range(2):
                    pr = ps1.tile([128, G], F32, tag="psa")
                    pi = ps1.tile([128, G], F32, tag="psa")
                    for xt in range(2):
                        mrb = mt[xt][:, yt * 128:yt * 128 + 128]
                        mib = mt[xt][:, IM + yt * 128:IM + yt * 128 + 128]
                        st = xt == 0
                        sp = xt == 1
                        nc.tensor.matmul(pr[:], mrb, artT[xt][:], start=st, stop=False)
                        nc.tensor.matmul(pi[:], mrb, aitT[xt][:], start=st, stop=False)
                        nc.tensor.matmul(pr[:], mib, aitnT[xt][:], start=False, stop=sp)
                        nc.tensor.matmul(pi[:], mib, artT[xt][:], start=False, stop=sp)
                    btr = btp.tile([128, G], BF16, tag=f"bt{c}r{yt}")
                    bti = btp.tile([128, G], BF16, tag=f"bt{c}i{yt}")
                    nc.scalar.copy(out=btr[:], in_=pr[:])
                    nc.vector.tensor_copy(out=bti[:], in_=pi[:])
                    bts[(0, yt, c)] = btr
                    bts[(1, yt, c)] = bti

            # ======== stage 2 vt-major; stores released per vt ========
            store_groups = {"vt0": [], "vt1": [], "vt2": [], "vt3": [],
                            "h0": [], "h3": []}
            T4v = T4[:].rearrange("(r c u) e -> r c (u e)", c=NCOPY, u=UPR)
            for vt in (3, 0, 1, 2):
                stg = stgs[vt]
                stg3 = stg[:].rearrange("p (u e) -> p u e", e=CELL)
                for c in range(NC):
                    gr = ps2.tile([128, G], F32, tag="psb")
                    gi = ps2.tile([128, G], F32, tag="psb")
                    for yt in range(2):
                        av = artT[yt][:, vt * 128:(vt + 1) * 128]
                        aiv = aitT[yt][:, vt * 128:(vt + 1) * 128]
                        ainv = aitnT[yt][:, vt * 128:(vt + 1) * 128]
                        btr = bts[(0, yt, c)]
                        bti = bts[(1, yt, c)]
                        st = yt == 0
                        sp = yt == 1
                        nc.tensor.matmul(gr[:], av, btr[:], start=st, stop=False)
                        nc.tensor.matmul(gi[:], aiv, btr[:], start=st, stop=False)
                        nc.tensor.matmul(gr[:], ainv, bti[:], start=False, stop=sp)
                        nc.tensor.matmul(gi[:], av, bti[:], start=False, stop=sp)
                    nc.scalar.copy(
                        out=stg3[:, 2:2 + G, 2 * c:2 * c + 1], in_=gr[:].unsqueeze(2)
                    )
                    if c < 2:
                        nc.scalar.copy(
                            out=stg3[:, 2:2 + G, 2 * c + 1:2 * c + 2],
                            in_=gi[:].unsqueeze(2),
                        )
                    else:
                        nc.vector.tensor_copy(
                            out=stg3[:, 2:2 + G, 2 * c + 1:2 * c + 2],
                            in_=gi[:].unsqueeze(2),
                        )
                # wrap halo cells, then store 4 shifted copies
                nc.vector.tensor_copy(
                    out=stg[:, 0:2 * CELL], in_=stg[:, 512 * CELL:514 * CELL]
                )
                nc.vector.tensor_copy(
                    out=stg[:, 514 * CELL:526 * CELL], in_=stg[:, 2 * CELL:14 * CELL]
                )
                r0 = vt * 128 + 2
                for cc in range(NCOPY):
                    store_groups[f"vt{vt}"].append(nc.sync.dma_start(
                        out=T4v[r0:r0 + 128, cc, :],
                        in_=stg[:, 2 * cc * CELL:(2 * cc + 520) * CELL],
                    ))
                if vt == 0:
                    for cc in range(NCOPY):
                        store_groups["h0"].append(nc.sync.dma_start(
                            out=T4v[514:517, cc, :],
                            in_=stg[0:3, 2 * cc * CELL:(2 * cc + 520) * CELL],
                        ))
                if vt == 3:
                    for cc in range(NCOPY):
                        store_groups["h3"].append(nc.sync.dma_start(
                            out=T4v[0:2, cc, :],
                            in_=stg[126:128, 2 * cc * CELL:(2 * cc + 520) * CELL],
                        ))

            # ---------------- w load + sqrt ----------------
            wsq = workp.tile([128, NTILE * 128], F32, tag="wsq")
            nc.sync.dma_start(out=wsq[:], in_=w_in[:])
            nc.scalar.activation(
                out=wsq[:], in_=wsq[:],
                func=mybir.ActivationFunctionType.Sqrt,
            )

            # ---------------- idx / meta loads ----------------
            idx_rep = workp.tile([128, NTILE * NB * 64], I16, tag="idxrep")
            nc.sync.dma_start(out=idx_rep[:], in_=i_in[:])
            meta_all = workp.tile([128, NTILE * 8 * 2], F32, tag="meta")
            nc.sync.dma_start(out=meta_all[:], in_=m_in[:])

            # ======== weights (all tiles) ========
            exv = bass.AP(
                meta_all[:].tensor, meta_all[:].offset,
                [meta_all[:].ap[0], [2, NTILE * 8], [0, NS]],
            )
            sconst = constp.tile([128, NS], F32, tag="sconst")
            for s_ in range(NS):
                nc.vector.memset(sconst[:, s_:s_ + 1], float(2 - s_))
            ux = workp.tile([128, NTILE * 8 * NS], F32, tag="ux")
            nc.vector.tensor_tensor(
                out=ux[:].rearrange("p (mc s) -> p mc s", s=NS),
                in0=exv, in1=bass.AP(
                    sconst[:].tensor, sconst[:].offset,
                    [sconst[:].ap[0], [0, NTILE * 8], [1, NS]],
                ),
                op=OP.add,
            )
            nc.vector.tensor_mul(ux[:], ux[:], ux[:])
            nc.vector.tensor_scalar(
                out=ux[:], in0=ux[:], scalar1=float(-1.0 / 9.0), scalar2=1.0,
                op0=OP.mult, op1=OP.add,
            )
            nc.vector.tensor_scalar_max(ux[:], ux[:], 0.0)
            wxall = horner_w(workp, ux[:], [128, NTILE * 8 * NS], "wxall")

            eyv = bass.AP(
                meta_all[:].tensor, meta_all[:].offset + 1,
                [meta_all[:].ap[0], [2, NTILE * 8], [0, NB]],
            )
            bconst = constp.tile([128, NB], F32, tag="bconst")
            for b in range(NB):
                nc.vector.memset(bconst[:, b:b + 1], float(2 - b))
            uy = workp.tile([128, NTILE * 8 * NB], F32, tag="uy")
            nc.vector.tensor_tensor(
                out=uy[:].rearrange("p (mc b) -> p mc b", b=NB),
                in0=eyv, in1=bass.AP(
                    bconst[:].tensor, bconst[:].offset,
                    [bconst[:].ap[0], [0, NTILE * 8], [1, NB]],
                ),
                op=OP.add,
            )
            nc.vector.tensor_mul(uy[:], uy[:], uy[:])
            nc.vector.tensor_scalar(
                out=uy[:], in0=uy[:], scalar1=float(-1.0 / 9.0), scalar2=1.0,
                op0=OP.mult, op1=OP.add,
            )
            nc.vector.tensor_scalar_max(uy[:], uy[:], 0.0)
            wyall = horner_w(workp, uy[:], [128, NTILE * 8 * NB], "wyall")


            # ======== gather + combine (release order _MORDER) ========
            for m in _MORDER:
                patch = patchp.tile([128, NB * 8 * 128], BF16, tag="patch")
                gathers = []
                for b in range(NB):
                    gi_ = nc.gpsimd.dma_gather(
                        out_ap=patch[:, b * 1024:(b + 1) * 1024].rearrange(
                            "p (ch e) -> p ch e", e=128),
                        in_ap=T4[_base_unit(m):_base_unit(m) + WIN, :],
                        idxs_ap=idx_rep[:, (m * NB + b) * 64:(m * NB + b + 1) * 64],
                        num_idxs=1024,
                        num_idxs_reg=1024,
                        elem_size=128,
                        queue_num=(m * NB + b) % 4,
                    )
                    gathers.append(gi_)
                for gi_ in gathers:
                    for grp in _NEEDS[m]:
                        for si in store_groups[grp]:
                            tile.add_dep_helper(gi_.ins, si.ins, reason="T RAW")
                # x-weights read directly (C, s, cr-broadcast) like the ymult
                wxs = bass.AP(
                    wxall[:].tensor, wxall[:].offset + m * 8 * NS,
                    [wxall[:].ap[0], [NS, 8], [1, NS], [0, CELL]],
                )
                rb = cmbp.tile([128, NB * 128], F32, tag="rb")
                for b in range(NB):
                    wp_ = cmbp.tile([128, 1024], BF16, tag="wp")
                    nc.vector.tensor_tensor(
                        out=wp_[:].rearrange("p (c s e) -> p c s e", s=NS, e=CELL),
                        in0=patch[:, b * 1024:(b + 1) * 1024].rearrange(
                            "p (c s e) -> p c s e", s=NS, e=CELL),
                        in1=wxs, op=OP.mult,
                    )
                    # tree-add s-reduce (contiguous 64/32-elem runs)
                    h1 = cmbp.tile([128, 512], BF16, tag="h1")
                    v0 = bass.AP(wp_[:].tensor, wp_[:].offset,
                                 [wp_[:].ap[0], [128, 8], [1, 64]])
                    v1 = bass.AP(wp_[:].tensor, wp_[:].offset + 64,
                                 [wp_[:].ap[0], [128, 8], [1, 64]])
                    h1v = h1[:].rearrange("p (ch e) -> p ch e", e=64)
                    nc.vector.tensor_tensor(out=h1v, in0=v0, in1=v1, op=OP.add)
                    h2 = cmbp.tile([128, 256], BF16, tag="h2")
                    w0 = bass.AP(h1[:].tensor, h1[:].offset,
                                 [h1[:].ap[0], [64, 8], [1, 32]])
                    w1 = bass.AP(h1[:].tensor, h1[:].offset + 32,
                                 [h1[:].ap[0], [64, 8], [1, 32]])
                    h2v = h2[:].rearrange("p (ch e) -> p ch e", e=32)
                    nc.vector.tensor_tensor(out=h2v, in0=w0, in1=w1, op=OP.add)
                    rb3 = rb[:, b * 128:(b + 1) * 128].rearrange(
                        "p (ch e) -> p ch e", e=16)
                    z0 = bass.AP(h2[:].tensor, h2[:].offset,
                                 [h2[:].ap[0], [32, 8], [1, 16]])
                    z1 = bass.AP(h2[:].tensor, h2[:].offset + 16,
                                 [h2[:].ap[0], [32, 8], [1, 16]])
                    nc.vector.tensor_tensor(out=rb3, in0=z0, in1=z1, op=OP.add)
                # vb[P, (b, C, cr)] = rb * wy ; tree-add over b
                vb = cmbp.tile([128, NB * 128], F32, tag="vb")
                rbv = bass.AP(
                    rb[:].tensor, rb[:].offset,
                    [rb[:].ap[0], [128, NB], [CELL, 8], [1, CELL]],
                )
                wys = bass.AP(
                    wyall[:].tensor, wyall[:].offset + m * 8 * NB,
                    [wyall[:].ap[0], [1, NB], [NB, 8], [0, CELL]],
                )
                nc.vector.tensor_tensor(
                    out=vb[:].rearrange("p (b ch e) -> p b ch e", ch=8, e=CELL),
                    in0=rbv, in1=wys, op=OP.mult,
                )
                h3_ = cmbp.tile([128, 384], F32, tag="h3t")
                nc.vector.tensor_tensor(
                    out=h3_[:], in0=vb[:, 0:384], in1=vb[:, 384:768], op=OP.add
                )
                h4 = cmbp.tile([128, 128], F32, tag="h4t")
                nc.vector.tensor_tensor(
                    out=h4[:], in0=h3_[:, 0:128], in1=h3_[:, 128:256], op=OP.add
                )
                nc.vector.tensor_tensor(
                    out=res[:, m * 128:(m + 1) * 128], in0=h4[:],
                    in1=h3_[:, 256:384], op=OP.add,
                )

            # ======== sqrt(w) scale + store ========
            nc.vector.tensor_mul(res[:], res[:], wsq[:])
            nc.sync.dma_start(out=y_out[:], in_=res[:])

    nc.compile()
    return nc


_NC_CACHE = None


def _get_nc():
    global _NC_CACHE
    if _NC_CACHE is None:
        _NC_CACHE = build_bass()
    return _NC_CACHE


# ---------------------------------------------------------------- host glue
def _point_map():
    P = np.arange(128)
    m = np.arange(NTILE)
    C = np.arange(8)
    return (m[None, :, None] * 1024 + (P % 16)[:, None, None] * 64
            + C[None, None, :] * 8 + (P // 16)[:, None, None])


_PMAP = _point_map()
_BASES = np.array([_base_unit(m) for m in range(NTILE)], dtype=np.int64)


def _host_idx_meta(kt):
    kv = np.asarray(kt, dtype=np.float32)
    gx0 = kv * np.float32(G / TWO_PI)
    gxy = np.where(gx0 < 0, gx0 + np.float32(G), gx0).astype(np.float32)
    gm3 = (gxy - np.float32(3.0)).astype(np.float32)
    fl = np.round((gm3 - np.float32(0.498046875)).astype(np.float32))
    rr = (gm3 - fl).astype(np.float32)
    fli = fl.astype(np.int64)
    perm = np.argsort(fli[1], kind="stable").astype(np.int64)

    q0 = 3 + fli[0]
    u8, m8 = q0 // 8, q0 % 8
    c4, d2 = m8 // 2, m8 % 2
    xunit = 65 * c4 + u8
    row0 = fli[1] + 3

    sp = perm
    m_of_s = np.arange(K) // 1024
    flat0 = 260 * row0[sp] + xunit[sp] - _BASES[m_of_s]
    s_grid = (np.arange(16)[:, None, None] * 64
              + np.arange(NTILE)[None, :, None] * 1024
              + np.arange(64)[None, None, :])
    f0 = flat0[s_grid]
    b_off = (260 * np.arange(NB))[None, :, None]
    idxw = f0[:, :, None, :] + b_off[None]
    idxw = np.clip(idxw, 0, WIN - 1).astype(np.int16)
    idxw = idxw.reshape(16, NTILE * NB * 64)
    idx_rep = np.ascontiguousarray(np.tile(idxw, (8, 1)))

    kmap = perm[_PMAP]
    ex = (rr[0] + d2.astype(np.float32))[kmap]
    ey = rr[1][kmap]
    meta = np.stack([ex, ey], axis=-1).astype(np.float32)
    return perm, idx_rep, np.ascontiguousarray(meta.reshape(128, NTILE * 8 * 2))


def make_in_maps(x, k, coil_sensitivities, w):
    in_maps = []
    coil0 = np.ascontiguousarray(coil_sensitivities[0], dtype=np.float32)
    perms = []
    for t in range(NT):
        perm, idx_rep, meta = _host_idx_meta(np.asarray(k[t], dtype=np.float32))
        perms.append(perm)
        kmap = perm[_PMAP]
        wt = np.asarray(w[t], dtype=np.float32)
        wr = wt[:, :, kmap]
        wr = np.ascontiguousarray(
            wr.transpose(2, 3, 4, 0, 1).reshape(128, NTILE * 128))
        in_maps.append({
            "x": np.ascontiguousarray(x[t], dtype=np.float32),
            "coil": coil0,
            "wr": wr,
            "idxr": idx_rep,
            "meta": meta,
            "art": _ARTB, "ait": _AITB, "aitn": _AITNB,
        })
    return in_maps, perms


def _unshuffle_y(yr, perm):
    v = yr.reshape(128, NTILE, 8, NC, 2)
    kmap = perm[_PMAP]
    out = np.empty((NC, 2, K), dtype=np.float32)
    out[:, :, kmap] = v.transpose(3, 4, 0, 1, 2)
    return out


def run(x, k, coil_sensitivities, w, trace=False, **spmd_kwargs):
    nc = _get_nc()
    in_maps, perms = make_in_maps(x, k, coil_sensitivities, w)
    r = run_bass_kernel_spmd(nc, in_maps, list(range(NT)), trace=trace, **spmd_kwargs)
    y = np.stack(
        [_unshuffle_y(r.results[t]["yr"], perms[t]) for t in range(NT)], axis=0
    )
    return y.astype(np.float32), r


def kernel(x, k, coil_sensitivities, w):
    y, _ = run(x, k, coil_sensitivities, w, trace=False)
    return y
